# revision 1
# baseline (speedup 1.0000x reference)
"""ALiBi attention-score kernel for 8 TRN2 NeuronCores.

Computes  out[b,h,i,j] = (q[b,h,i,:] * head_scales[h] / sqrt(D)) . k[b,h,j,:]
                         - slopes[h] * (pos[b,i] - pos[b,j])
with pos = positions[token_indices], for B=2, H=16, S=2048, D=128.

Sharding: the 32 (b,h) pairs are split 4-per-core across 8 cores (batch+head
parallel, no cross-core communication). Host prep transposes q/k to [D, S]
bf16 (d on partitions, ready as matmul operands) and folds the whole ALiBi
bias into 4 extra bf16 contraction rows (hi/lo split of s*pos for ~f32
accuracy), so the device does only: matmul(K=128) + matmul(K=4) -> PSUM,
PSUM->SBUF copy (split across ScalarE/VectorE), 1 MiB DMA stores.
"""
import sys

if "/opt/trn_rl_repo" not in sys.path:
    sys.path.insert(0, "/opt/trn_rl_repo")

import math

import numpy as np
import ml_dtypes

import concourse.bacc as bacc
import concourse.mybir as mybir
import concourse.tile as tile
from concourse.bass_utils import run_bass_kernel_spmd

B, H, S, D = 2, 16, 2048, 128
N_CORES = 8
PAIRS_PER_CORE = (B * H) // N_CORES  # 4
QT = S // 128   # 16 q-tiles of 128 rows
NC_CHUNK = 512  # matmul free-dim (one PSUM bank)
NCH = S // NC_CHUNK  # 4

BF16 = mybir.dt.bfloat16
F32 = mybir.dt.float32

_compiled_nc = None


def _build_nc():
    nc = bacc.Bacc("TRN2", target_bir_lowering=False, debug=False,
                   num_devices=N_CORES)
    qT = nc.dram_tensor("qT", [PAIRS_PER_CORE, D, S], BF16, kind="ExternalInput")
    kT = nc.dram_tensor("kT", [PAIRS_PER_CORE, D, S], BF16, kind="ExternalInput")
    bL = nc.dram_tensor("bL", [PAIRS_PER_CORE, 4, S], BF16, kind="ExternalInput")
    bR = nc.dram_tensor("bR", [PAIRS_PER_CORE, 4, S], BF16, kind="ExternalInput")
    out = nc.dram_tensor("out", [PAIRS_PER_CORE, S, S], F32, kind="ExternalOutput")

    with tile.TileContext(nc) as tc:
        with (
            tc.tile_pool(name="qpool", bufs=2) as qpool,
            tc.tile_pool(name="kpool", bufs=2) as kpool,
            tc.tile_pool(name="bpool", bufs=2) as bpool,
            tc.tile_pool(name="opool", bufs=4) as opool,
            tc.tile_pool(name="psum", bufs=2, space="PSUM") as psum_pool,
        ):
            for u in range(PAIRS_PER_CORE):
                q_t = qpool.tile([D, S], BF16, tag="q")
                k_t = kpool.tile([D, S], BF16, tag="k")
                bl_t = bpool.tile([4, S], BF16, tag="bl")
                br_t = bpool.tile([4, S], BF16, tag="br")
                nc.scalar.dma_start(q_t[:], qT[u])
                nc.scalar.dma_start(k_t[:], kT[u])
                nc.scalar.dma_start(bl_t[:], bL[u])
                nc.scalar.dma_start(br_t[:], bR[u])

                for qt in range(QT):
                    ps = psum_pool.tile([128, S], F32, tag="ps")
                    for n in range(NCH):
                        nc.tensor.matmul(
                            ps[:, n * NC_CHUNK:(n + 1) * NC_CHUNK],
                            q_t[:, qt * 128:(qt + 1) * 128],
                            k_t[:, n * NC_CHUNK:(n + 1) * NC_CHUNK],
                            start=True, stop=False,
                        )
                    for n in range(NCH):
                        nc.tensor.matmul(
                            ps[:, n * NC_CHUNK:(n + 1) * NC_CHUNK],
                            bl_t[:, qt * 128:(qt + 1) * 128],
                            br_t[:, n * NC_CHUNK:(n + 1) * NC_CHUNK],
                            start=False, stop=True,
                        )
                    o_t = opool.tile([128, S], F32, tag="o")
                    nc.scalar.copy(o_t[:, 0:1024], ps[:, 0:1024])
                    nc.vector.tensor_copy(o_t[:, 1024:2048], ps[:, 1024:2048])
                    nc.sync.dma_start(out[u, qt * 128:(qt + 1) * 128, :], o_t[:])

    nc.compile()
    return nc


def _get_nc():
    global _compiled_nc
    if _compiled_nc is None:
        _compiled_nc = _build_nc()
    return _compiled_nc


def kernel(q, k, head_scales, slopes, positions, token_indices, **_unused):
    q = np.asarray(q, dtype=np.float32)
    k = np.asarray(k, dtype=np.float32)
    head_scales = np.asarray(head_scales, dtype=np.float32)
    slopes = np.asarray(slopes, dtype=np.float32)
    positions = np.asarray(positions, dtype=np.float32)
    token_indices = np.asarray(token_indices)

    base_scale = 1.0 / math.sqrt(D)
    # [B, S] gathered positions, then r = slope * pos per (b, h)
    pos = positions[token_indices]                              # [B, S] f32
    r = slopes[None, :, None] * pos[:, None, :]                 # [B, H, S] f32
    r_hi = r.astype(ml_dtypes.bfloat16)
    r_lo = (r - r_hi.astype(np.float32)).astype(ml_dtypes.bfloat16)

    # q scaled per head, then [B,H,D,S] bf16 (d on partitions)
    q_scaled = q * (head_scales * base_scale)[None, :, None, None]
    qT = np.ascontiguousarray(np.swapaxes(q_scaled, -1, -2)).astype(
        ml_dtypes.bfloat16)                                     # [B,H,D,S]
    kT = np.ascontiguousarray(np.swapaxes(k, -1, -2)).astype(
        ml_dtypes.bfloat16)                                     # [B,H,D,S]

    ones = np.ones((B, H, S), dtype=ml_dtypes.bfloat16)
    # bias rows: out += bL.T @ bR with
    #   bL rows = [-r_hi(q_pos), -r_lo(q_pos), 1, 1]   (K=4, M=S)
    #   bR rows = [ 1,            1,  r_hi(k_pos), r_lo(k_pos)] (K=4, N=S)
    bL = np.stack([-r_hi, -r_lo, ones, ones], axis=2)           # [B,H,4,S]
    bR = np.stack([ones, ones, r_hi, r_lo], axis=2)             # [B,H,4,S]

    qT = qT.reshape(B * H, D, S)
    kT = kT.reshape(B * H, D, S)
    bL = bL.reshape(B * H, 4, S)
    bR = bR.reshape(B * H, 4, S)

    in_maps = []
    for c in range(N_CORES):
        sl = slice(c * PAIRS_PER_CORE, (c + 1) * PAIRS_PER_CORE)
        in_maps.append({
            "qT": np.ascontiguousarray(qT[sl]),
            "kT": np.ascontiguousarray(kT[sl]),
            "bL": np.ascontiguousarray(bL[sl]),
            "bR": np.ascontiguousarray(bR[sl]),
        })

    nc = _get_nc()
    res = run_bass_kernel_spmd(nc, in_maps, core_ids=list(range(N_CORES)))
    outs = [np.asarray(res.results[c]["out"], dtype=np.float32)
            for c in range(N_CORES)]
    full = np.concatenate(outs, axis=0).reshape(B, H, S, S)
    return full


if __name__ == "__main__":
    rng = np.random.default_rng(0)
    inputs = {
        "q": rng.standard_normal((B, H, S, D), dtype=np.float32),
        "k": rng.standard_normal((B, H, S, D), dtype=np.float32),
        "head_scales": np.full((H,), 1.2, dtype=np.float32),
        "slopes": (2.0 ** (-8.0 * np.arange(1, H + 1) / H)).astype(np.float32),
        "positions": np.arange(S, dtype=np.float32),
        "token_indices": np.sort(rng.integers(0, S, (B, S)).astype(np.int32), axis=-1),
    }
    out = kernel(**inputs)
    print("kernel output", out.shape, out.dtype)


# revision 2
# speedup vs baseline: 1.5006x; 1.5006x over previous
"""ALiBi attention-score kernel for 8 TRN2 NeuronCores.

Computes  out[b,h,i,j] = (q[b,h,i,:] * head_scales[h] / sqrt(D)) . k[b,h,j,:]
                         - slopes[h] * (pos[b,i] - pos[b,j])
with pos = positions[token_indices], for B=2, H=16, S=2048, D=128.

Sharding: the 32 (b,h) pairs are split 4-per-core across 8 cores (batch+head
parallel, no cross-core communication).

Device dataflow per (b,h):
 - PE: scores matmul (K=128 bf16, N=512 chunks) into PSUM, plus one tiny
   K=2 matmul that builds R[p,j] = slope*pos_k[j] (hi/lo bf16 split of the
   row bias, replicated across partitions via a ones stationary operand).
 - ScalarE: PSUM->SBUF copy fused with the per-partition bias add
   (-slope*pos_q), writing f16; VectorE handles the tail columns via
   tensor_scalar_add, then adds R to the whole row block in f16 2x mode.
 - f16 output tiles (1 MiB per 2 q-tiles) DMA'd to DRAM; host upcasts to f32.
"""
import sys

if "/opt/trn_rl_repo" not in sys.path:
    sys.path.insert(0, "/opt/trn_rl_repo")

import math

import numpy as np
import ml_dtypes

import concourse.bacc as bacc
import concourse.mybir as mybir
import concourse.tile as tile
from concourse.bass_utils import run_bass_kernel_spmd

B, H, S, D = 2, 16, 2048, 128
N_CORES = 8
PAIRS_PER_CORE = (B * H) // N_CORES  # 4
QT = S // 128   # 16 q-tiles of 128 rows
NC_CHUNK = 512  # matmul free-dim (one PSUM bank)
NCH = S // NC_CHUNK  # 4
ACT_COLS = 1728  # columns of each q-tile handled by ScalarE (rest by VectorE)

BF16 = mybir.dt.bfloat16
F16 = mybir.dt.float16
F32 = mybir.dt.float32

_compiled_nc = None


def _build_nc():
    nc = bacc.Bacc("TRN2", target_bir_lowering=False, debug=False,
                   num_devices=N_CORES)
    qT = nc.dram_tensor("qT", [PAIRS_PER_CORE, D, S], BF16, kind="ExternalInput")
    kT = nc.dram_tensor("kT", [PAIRS_PER_CORE, D, S], BF16, kind="ExternalInput")
    rrow = nc.dram_tensor("rrow", [PAIRS_PER_CORE, 2, S], BF16,
                          kind="ExternalInput")
    pq = nc.dram_tensor("pq", [PAIRS_PER_CORE, 128, QT], F32,
                        kind="ExternalInput")
    out = nc.dram_tensor("out", [PAIRS_PER_CORE, S, S], F16,
                         kind="ExternalOutput")

    with tile.TileContext(nc) as tc:
        with (
            tc.tile_pool(name="const", bufs=1) as cpool,
            tc.tile_pool(name="qpool", bufs=2) as qpool,
            tc.tile_pool(name="kpool", bufs=2) as kpool,
            tc.tile_pool(name="spool", bufs=2) as spool,
            tc.tile_pool(name="rpool", bufs=2) as rpool,
            tc.tile_pool(name="opool", bufs=3) as opool,
            tc.tile_pool(name="psum", bufs=2, space="PSUM") as psum_pool,
        ):
            ones2 = cpool.tile([2, 128], BF16)
            nc.gpsimd.memset(ones2[:], 1.0)

            for u in range(PAIRS_PER_CORE):
                q_t = qpool.tile([D, S], BF16, tag="q")
                k_t = kpool.tile([D, S], BF16, tag="k")
                rr_t = spool.tile([2, S], BF16, tag="rr")
                pq_t = spool.tile([128, QT], F32, tag="pq")
                nc.sync.dma_start(q_t[:], qT[u])
                nc.sync.dma_start(k_t[:], kT[u])
                nc.sync.dma_start(rr_t[:], rrow[u])
                nc.sync.dma_start(pq_t[:], pq[u])

                # R[p, j] = slope*pos_k[j] (replicated over partitions):
                # ones2.T @ [r_hi; r_lo]
                ps_r = psum_pool.tile([128, S], F32, tag="ps")
                for n in range(NCH):
                    sl = slice(n * NC_CHUNK, (n + 1) * NC_CHUNK)
                    nc.tensor.matmul(ps_r[:, sl], ones2[:], rr_t[:, sl],
                                     start=True, stop=True)
                r16_t = rpool.tile([128, S], F16, tag="r16")
                nc.scalar.copy(r16_t[:], ps_r[:])

                out_v = out[u].rearrange("(blk p) c -> p blk c", p=128)

                for qt in range(QT):
                    ps = psum_pool.tile([128, S], F32, tag="ps")
                    for n in range(NCH):
                        sl = slice(n * NC_CHUNK, (n + 1) * NC_CHUNK)
                        nc.tensor.matmul(
                            ps[:, sl],
                            q_t[:, qt * 128:(qt + 1) * 128],
                            k_t[:, sl],
                            start=True, stop=True,
                        )
                    if qt % 2 == 0:
                        o16 = opool.tile([128, 2, S], F16, tag="o16")
                    half = qt % 2
                    colbias = pq_t[:, qt:qt + 1]
                    # PSUM -> SBUF with per-partition bias, f16 out
                    nc.scalar.activation(
                        o16[:, half, 0:ACT_COLS], ps[:, 0:ACT_COLS],
                        mybir.ActivationFunctionType.Identity,
                        bias=colbias, scale=1.0,
                    )
                    nc.vector.tensor_scalar_add(
                        o16[:, half, ACT_COLS:S], ps[:, ACT_COLS:S], colbias,
                    )
                    # += R  (f16 2x mode)
                    nc.vector.tensor_add(
                        o16[:, half, :], o16[:, half, :], r16_t[:],
                    )
                    if qt % 2 == 1:
                        nc.sync.dma_start(out_v[:, qt - 1:qt + 1, :], o16[:])

    nc.compile()
    return nc


def _get_nc():
    global _compiled_nc
    if _compiled_nc is None:
        _compiled_nc = _build_nc()
    return _compiled_nc


def kernel(q, k, head_scales, slopes, positions, token_indices, **_unused):
    q = np.asarray(q, dtype=np.float32)
    k = np.asarray(k, dtype=np.float32)
    head_scales = np.asarray(head_scales, dtype=np.float32)
    slopes = np.asarray(slopes, dtype=np.float32)
    positions = np.asarray(positions, dtype=np.float32)
    token_indices = np.asarray(token_indices)

    base_scale = 1.0 / math.sqrt(D)
    pos = positions[token_indices]                              # [B, S] f32
    r = slopes[None, :, None] * pos[:, None, :]                 # [B, H, S] f32
    r_hi = r.astype(ml_dtypes.bfloat16)
    r_lo = (r - r_hi.astype(np.float32)).astype(ml_dtypes.bfloat16)
    rrow = np.stack([r_hi, r_lo], axis=2)                       # [B, H, 2, S]

    # pq[b,h,p,qt] = -r[b,h, qt*128+p]
    pq = -np.swapaxes(r.reshape(B, H, QT, 128), -1, -2)         # [B, H, 128, QT]
    pq = np.ascontiguousarray(pq)

    q_scaled = q * (head_scales * base_scale)[None, :, None, None]
    qT = np.ascontiguousarray(np.swapaxes(q_scaled, -1, -2)).astype(
        ml_dtypes.bfloat16)                                     # [B,H,D,S]
    kT = np.ascontiguousarray(np.swapaxes(k, -1, -2)).astype(
        ml_dtypes.bfloat16)                                     # [B,H,D,S]

    qT = qT.reshape(B * H, D, S)
    kT = kT.reshape(B * H, D, S)
    rrow = rrow.reshape(B * H, 2, S)
    pq = pq.reshape(B * H, 128, QT)

    in_maps = []
    for c in range(N_CORES):
        sl = slice(c * PAIRS_PER_CORE, (c + 1) * PAIRS_PER_CORE)
        in_maps.append({
            "qT": np.ascontiguousarray(qT[sl]),
            "kT": np.ascontiguousarray(kT[sl]),
            "rrow": np.ascontiguousarray(rrow[sl]),
            "pq": np.ascontiguousarray(pq[sl]),
        })

    nc = _get_nc()
    res = run_bass_kernel_spmd(nc, in_maps, core_ids=list(range(N_CORES)))
    outs = [np.asarray(res.results[c]["out"]) for c in range(N_CORES)]
    full = np.concatenate(outs, axis=0).reshape(B, H, S, S).astype(np.float32)
    return full


if __name__ == "__main__":
    rng = np.random.default_rng(0)
    inputs = {
        "q": rng.standard_normal((B, H, S, D), dtype=np.float32),
        "k": rng.standard_normal((B, H, S, D), dtype=np.float32),
        "head_scales": np.full((H,), 1.2, dtype=np.float32),
        "slopes": (2.0 ** (-8.0 * np.arange(1, H + 1) / H)).astype(np.float32),
        "positions": np.arange(S, dtype=np.float32),
        "token_indices": np.sort(rng.integers(0, S, (B, S)).astype(np.int32), axis=-1),
    }
    out = kernel(**inputs)
    print("kernel output", out.shape, out.dtype)


# revision 7
# speedup vs baseline: 1.5859x; 1.0569x over previous
"""ALiBi attention-score kernel for 8 TRN2 NeuronCores.

Computes  out[b,h,i,j] = (q[b,h,i,:] * head_scales[h] / sqrt(D)) . k[b,h,j,:]
                         - slopes[h] * (pos[b,i] - pos[b,j])
with pos = positions[token_indices], for B=2, H=16, S=2048, D=128.

Sharding: the 32 (b,h) pairs are split 4-per-core across 8 cores (batch+head
parallel, no cross-core communication).

Device dataflow per (b,h):
 - PE: scores matmuls (K=128 bf16, N=512 chunks) into PSUM.
 - GpSimd: partition_broadcast builds R[p,j] = slope*pos_k[j] from a host
   f16 row (the +row part of the ALiBi bias).
 - Epilogue splits each q-tile's 2048 columns: the first V columns go
   through one fused VectorE op (affine_then_add: psum + colbias + R -> f16);
   the rest go ScalarE activation (psum + colbias) followed by a VectorE
   tensor_add of R in f16.
 - f16 output tiles (1 MiB per 2 q-tiles) DMA'd to DRAM; host upcasts to f32.
"""
import sys

if "/opt/trn_rl_repo" not in sys.path:
    sys.path.insert(0, "/opt/trn_rl_repo")

import math

import numpy as np
import ml_dtypes

import concourse.bacc as bacc
import concourse.mybir as mybir
import concourse.tile as tile
from concourse.bass_utils import run_bass_kernel_spmd

B, H, S, D = 2, 16, 2048, 128
N_CORES = 8
PAIRS_PER_CORE = (B * H) // N_CORES  # 4
QT = S // 128   # 16 q-tiles of 128 rows
NC_CHUNK = 512  # matmul free-dim (one PSUM bank)
NCH = S // NC_CHUNK  # 4
V_COLS = 640  # columns per q-tile via fused DVE affine_then_add; rest via ACT

BF16 = mybir.dt.bfloat16
F16 = mybir.dt.float16
F32 = mybir.dt.float32

_compiled_nc = None


def _build_nc():
    nc = bacc.Bacc("TRN2", target_bir_lowering=False, debug=False,
                   num_devices=N_CORES)
    qT = nc.dram_tensor("qT", [PAIRS_PER_CORE, D, S], BF16, kind="ExternalInput")
    kT = nc.dram_tensor("kT", [PAIRS_PER_CORE, D, S], BF16, kind="ExternalInput")
    rrow = nc.dram_tensor("rrow", [PAIRS_PER_CORE, 1, S], F16,
                          kind="ExternalInput")
    pq = nc.dram_tensor("pq", [PAIRS_PER_CORE, 128, QT], F32,
                        kind="ExternalInput")
    out = nc.dram_tensor("out", [PAIRS_PER_CORE, S, S], F16,
                         kind="ExternalOutput")

    with tile.TileContext(nc) as tc:
        with (
            tc.tile_pool(name="qpool", bufs=2) as qpool,
            tc.tile_pool(name="kpool", bufs=2) as kpool,
            tc.tile_pool(name="spool", bufs=2) as spool,
            tc.tile_pool(name="rpool", bufs=2) as rpool,
            tc.tile_pool(name="opool", bufs=3) as opool,
            tc.tile_pool(name="psum", bufs=2, space="PSUM") as psum_pool,
        ):
            for u in range(PAIRS_PER_CORE):
                q_t = qpool.tile([D, S], BF16, tag="q")
                k_t = kpool.tile([D, S], BF16, tag="k")
                rr_t = spool.tile([1, S], F16, tag="rr")
                pq_t = spool.tile([128, QT], F32, tag="pq")
                nc.sync.dma_start(q_t[:], qT[u])
                nc.sync.dma_start(k_t[:], kT[u])
                nc.sync.dma_start(rr_t[:], rrow[u])
                nc.sync.dma_start(pq_t[:], pq[u])

                # R[p, j] = slope*pos_k[j] replicated across partitions
                r16_t = rpool.tile([128, S], F16, tag="r16")
                nc.gpsimd.partition_broadcast(r16_t[:], rr_t[:])

                out_v = out[u].rearrange("(blk p) c -> p blk c", p=128)

                for qt in range(QT):
                    ps = psum_pool.tile([128, S], F32, tag="ps")
                    for n in range(NCH):
                        sl = slice(n * NC_CHUNK, (n + 1) * NC_CHUNK)
                        nc.tensor.matmul(
                            ps[:, sl],
                            q_t[:, qt * 128:(qt + 1) * 128],
                            k_t[:, sl],
                            start=True, stop=True,
                        )
                    if qt % 2 == 0:
                        o16 = opool.tile([128, 2, S], F16, tag="o16")
                    half = qt % 2
                    colbias = pq_t[:, qt:qt + 1]
                    # fused: (psum + colbias) + R -> f16, first V_COLS cols
                    nc.vector.affine_then_add(
                        o16[:, half, 0:V_COLS], ps[:, 0:V_COLS],
                        r16_t[:, 0:V_COLS], scale=1.0, bias=colbias,
                    )
                    # remaining cols: ACT does psum + colbias, DVE adds R
                    nc.scalar.activation(
                        o16[:, half, V_COLS:S], ps[:, V_COLS:S],
                        mybir.ActivationFunctionType.Identity,
                        bias=colbias, scale=1.0,
                    )
                    nc.vector.tensor_add(
                        o16[:, half, V_COLS:S], o16[:, half, V_COLS:S],
                        r16_t[:, V_COLS:S],
                    )
                    if qt % 2 == 1:
                        nc.sync.dma_start(out_v[:, qt - 1:qt + 1, :], o16[:])

    nc.compile()
    return nc


def _get_nc():
    global _compiled_nc
    if _compiled_nc is None:
        _compiled_nc = _build_nc()
    return _compiled_nc


def kernel(q, k, head_scales, slopes, positions, token_indices, **_unused):
    q = np.asarray(q, dtype=np.float32)
    k = np.asarray(k, dtype=np.float32)
    head_scales = np.asarray(head_scales, dtype=np.float32)
    slopes = np.asarray(slopes, dtype=np.float32)
    positions = np.asarray(positions, dtype=np.float32)
    token_indices = np.asarray(token_indices)

    base_scale = 1.0 / math.sqrt(D)
    pos = positions[token_indices]                              # [B, S] f32
    r = slopes[None, :, None] * pos[:, None, :]                 # [B, H, S] f32
    rrow = r.astype(np.float16)[:, :, None, :]                  # [B, H, 1, S]

    # pq[b,h,p,qt] = -r[b,h, qt*128+p]
    pq = -np.swapaxes(r.reshape(B, H, QT, 128), -1, -2)         # [B, H, 128, QT]
    pq = np.ascontiguousarray(pq)

    q_scaled = q * (head_scales * base_scale)[None, :, None, None]
    qT = np.ascontiguousarray(np.swapaxes(q_scaled, -1, -2)).astype(
        ml_dtypes.bfloat16)                                     # [B,H,D,S]
    kT = np.ascontiguousarray(np.swapaxes(k, -1, -2)).astype(
        ml_dtypes.bfloat16)                                     # [B,H,D,S]

    qT = qT.reshape(B * H, D, S)
    kT = kT.reshape(B * H, D, S)
    rrow = rrow.reshape(B * H, 1, S)
    pq = pq.reshape(B * H, 128, QT)

    in_maps = []
    for c in range(N_CORES):
        sl = slice(c * PAIRS_PER_CORE, (c + 1) * PAIRS_PER_CORE)
        in_maps.append({
            "qT": np.ascontiguousarray(qT[sl]),
            "kT": np.ascontiguousarray(kT[sl]),
            "rrow": np.ascontiguousarray(rrow[sl]),
            "pq": np.ascontiguousarray(pq[sl]),
        })

    nc = _get_nc()
    res = run_bass_kernel_spmd(nc, in_maps, core_ids=list(range(N_CORES)))
    outs = [np.asarray(res.results[c]["out"]) for c in range(N_CORES)]
    full = np.concatenate(outs, axis=0).reshape(B, H, S, S).astype(np.float32)
    return full


if __name__ == "__main__":
    rng = np.random.default_rng(0)
    inputs = {
        "q": rng.standard_normal((B, H, S, D), dtype=np.float32),
        "k": rng.standard_normal((B, H, S, D), dtype=np.float32),
        "head_scales": np.full((H,), 1.2, dtype=np.float32),
        "slopes": (2.0 ** (-8.0 * np.arange(1, H + 1) / H)).astype(np.float32),
        "positions": np.arange(S, dtype=np.float32),
        "token_indices": np.sort(rng.integers(0, S, (B, S)).astype(np.int32), axis=-1),
    }
    out = kernel(**inputs)
    print("kernel output", out.shape, out.dtype)


# revision 9
# speedup vs baseline: 1.5885x; 1.0016x over previous
"""ALiBi attention-score kernel for 8 TRN2 NeuronCores.

Computes  out[b,h,i,j] = (q[b,h,i,:] * head_scales[h] / sqrt(D)) . k[b,h,j,:]
                         - slopes[h] * (pos[b,i] - pos[b,j])
with pos = positions[token_indices], for B=2, H=16, S=2048, D=128.

Sharding: the 32 (b,h) pairs are split 4-per-core across 8 cores (batch+head
parallel, no cross-core communication).

Device dataflow per (b,h):
 - PE: scores matmuls (K=128 bf16, N=512 chunks) into PSUM.
 - GpSimd: partition_broadcast builds R[p,j] = slope*pos_k[j] from a host
   f16 row (the +row part of the ALiBi bias).
 - Epilogue splits each q-tile's 2048 columns: the first V columns go
   through one fused VectorE op (affine_then_add: psum + colbias + R -> f16);
   the rest go ScalarE activation (psum + colbias) followed by a VectorE
   tensor_add of R in f16.
 - f16 output tiles (1 MiB per 2 q-tiles) DMA'd to DRAM; host upcasts to f32.
"""
import sys

if "/opt/trn_rl_repo" not in sys.path:
    sys.path.insert(0, "/opt/trn_rl_repo")

import math

import numpy as np
import ml_dtypes

import concourse.bacc as bacc
import concourse.mybir as mybir
import concourse.tile as tile
from concourse.bass_utils import run_bass_kernel_spmd

B, H, S, D = 2, 16, 2048, 128
N_CORES = 8
PAIRS_PER_CORE = (B * H) // N_CORES  # 4
QT = S // 128   # 16 q-tiles of 128 rows
NC_CHUNK = 512  # matmul free-dim (one PSUM bank)
NCH = S // NC_CHUNK  # 4
V_COLS = 448  # columns per q-tile via fused DVE affine_then_add; rest via ACT

BF16 = mybir.dt.bfloat16
F16 = mybir.dt.float16
F32 = mybir.dt.float32

_compiled_nc = None


def _build_nc():
    nc = bacc.Bacc("TRN2", target_bir_lowering=False, debug=False,
                   num_devices=N_CORES)
    qT = nc.dram_tensor("qT", [PAIRS_PER_CORE, D, S], BF16, kind="ExternalInput")
    kT = nc.dram_tensor("kT", [PAIRS_PER_CORE, D, S], BF16, kind="ExternalInput")
    rrow = nc.dram_tensor("rrow", [PAIRS_PER_CORE, 1, S], F16,
                          kind="ExternalInput")
    pq = nc.dram_tensor("pq", [PAIRS_PER_CORE, 128, QT], F32,
                        kind="ExternalInput")
    out = nc.dram_tensor("out", [PAIRS_PER_CORE, S, S], F16,
                         kind="ExternalOutput")

    with tile.TileContext(nc) as tc:
        with (
            tc.tile_pool(name="qpool", bufs=2) as qpool,
            tc.tile_pool(name="kpool", bufs=2) as kpool,
            tc.tile_pool(name="spool", bufs=2) as spool,
            tc.tile_pool(name="rpool", bufs=2) as rpool,
            tc.tile_pool(name="opool", bufs=3) as opool,
            tc.tile_pool(name="psum", bufs=2, space="PSUM") as psum_pool,
        ):
            for u in range(PAIRS_PER_CORE):
                q_t = qpool.tile([D, S], BF16, tag="q")
                k_t = kpool.tile([D, S], BF16, tag="k")
                rr_t = spool.tile([1, S], F16, tag="rr")
                pq_t = spool.tile([128, QT], F32, tag="pq")
                nc.sync.dma_start(q_t[:], qT[u])
                nc.sync.dma_start(k_t[:], kT[u])
                nc.sync.dma_start(rr_t[:], rrow[u])
                nc.sync.dma_start(pq_t[:], pq[u])

                # R[p, j] = slope*pos_k[j] replicated across partitions
                r16_t = rpool.tile([128, S], F16, tag="r16")
                nc.gpsimd.partition_broadcast(r16_t[:], rr_t[:])

                out_v = out[u].rearrange("(blk p) c -> p blk c", p=128)

                # epilogue part 2 (R-add on the ACT region + store) is
                # emitted one q-tile late so DVE never stalls on ACT
                pending = None
                for qt in range(QT):
                    ps = psum_pool.tile([128, S], F32, tag="ps")
                    for n in range(NCH):
                        sl = slice(n * NC_CHUNK, (n + 1) * NC_CHUNK)
                        nc.tensor.matmul(
                            ps[:, sl],
                            q_t[:, qt * 128:(qt + 1) * 128],
                            k_t[:, sl],
                            start=True, stop=True,
                        )
                    if qt % 2 == 0:
                        o16 = opool.tile([128, 2, S], F16, tag="o16")
                    half = qt % 2
                    colbias = pq_t[:, qt:qt + 1]
                    # fused: (psum + colbias) + R -> f16, first V_COLS cols
                    nc.vector.affine_then_add(
                        o16[:, half, 0:V_COLS], ps[:, 0:V_COLS],
                        r16_t[:, 0:V_COLS], scale=1.0, bias=colbias,
                    )
                    # remaining cols: ACT does psum + colbias, DVE adds R
                    nc.scalar.activation(
                        o16[:, half, V_COLS:S], ps[:, V_COLS:S],
                        mybir.ActivationFunctionType.Identity,
                        bias=colbias, scale=1.0,
                    )
                    if pending is not None:
                        p_o16, p_half, p_store, p_r16 = pending
                        nc.vector.tensor_add(
                            p_o16[:, p_half, V_COLS:S],
                            p_o16[:, p_half, V_COLS:S],
                            p_r16[:, V_COLS:S],
                        )
                        if p_store is not None:
                            nc.sync.dma_start(p_store, p_o16[:])
                    store = out_v[:, qt - 1:qt + 1, :] if qt % 2 == 1 else None
                    pending = (o16, half, store, r16_t)
                if pending is not None:
                    p_o16, p_half, p_store, p_r16 = pending
                    nc.vector.tensor_add(
                        p_o16[:, p_half, V_COLS:S],
                        p_o16[:, p_half, V_COLS:S],
                        p_r16[:, V_COLS:S],
                    )
                    if p_store is not None:
                        nc.sync.dma_start(p_store, p_o16[:])

    nc.compile()
    return nc


def _get_nc():
    global _compiled_nc
    if _compiled_nc is None:
        _compiled_nc = _build_nc()
    return _compiled_nc


def kernel(q, k, head_scales, slopes, positions, token_indices, **_unused):
    q = np.asarray(q, dtype=np.float32)
    k = np.asarray(k, dtype=np.float32)
    head_scales = np.asarray(head_scales, dtype=np.float32)
    slopes = np.asarray(slopes, dtype=np.float32)
    positions = np.asarray(positions, dtype=np.float32)
    token_indices = np.asarray(token_indices)

    base_scale = 1.0 / math.sqrt(D)
    pos = positions[token_indices]                              # [B, S] f32
    r = slopes[None, :, None] * pos[:, None, :]                 # [B, H, S] f32
    rrow = r.astype(np.float16)[:, :, None, :]                  # [B, H, 1, S]

    # pq[b,h,p,qt] = -r[b,h, qt*128+p]
    pq = -np.swapaxes(r.reshape(B, H, QT, 128), -1, -2)         # [B, H, 128, QT]
    pq = np.ascontiguousarray(pq)

    q_scaled = q * (head_scales * base_scale)[None, :, None, None]
    qT = np.ascontiguousarray(np.swapaxes(q_scaled, -1, -2)).astype(
        ml_dtypes.bfloat16)                                     # [B,H,D,S]
    kT = np.ascontiguousarray(np.swapaxes(k, -1, -2)).astype(
        ml_dtypes.bfloat16)                                     # [B,H,D,S]

    qT = qT.reshape(B * H, D, S)
    kT = kT.reshape(B * H, D, S)
    rrow = rrow.reshape(B * H, 1, S)
    pq = pq.reshape(B * H, 128, QT)

    in_maps = []
    for c in range(N_CORES):
        sl = slice(c * PAIRS_PER_CORE, (c + 1) * PAIRS_PER_CORE)
        in_maps.append({
            "qT": np.ascontiguousarray(qT[sl]),
            "kT": np.ascontiguousarray(kT[sl]),
            "rrow": np.ascontiguousarray(rrow[sl]),
            "pq": np.ascontiguousarray(pq[sl]),
        })

    nc = _get_nc()
    res = run_bass_kernel_spmd(nc, in_maps, core_ids=list(range(N_CORES)))
    outs = [np.asarray(res.results[c]["out"]) for c in range(N_CORES)]
    full = np.concatenate(outs, axis=0).reshape(B, H, S, S).astype(np.float32)
    return full


if __name__ == "__main__":
    rng = np.random.default_rng(0)
    inputs = {
        "q": rng.standard_normal((B, H, S, D), dtype=np.float32),
        "k": rng.standard_normal((B, H, S, D), dtype=np.float32),
        "head_scales": np.full((H,), 1.2, dtype=np.float32),
        "slopes": (2.0 ** (-8.0 * np.arange(1, H + 1) / H)).astype(np.float32),
        "positions": np.arange(S, dtype=np.float32),
        "token_indices": np.sort(rng.integers(0, S, (B, S)).astype(np.int32), axis=-1),
    }
    out = kernel(**inputs)
    print("kernel output", out.shape, out.dtype)
